# revision 30
# baseline (speedup 1.0000x reference)
"""Trainium2 Bass kernel for nn_LocalTrans (gnn message passing).

Math (reference, with exact simplifications):
  k = f@kw + kb ; v = f@vw + vb            (per batch cloud)
  kg, vg = gather(k, idx), gather(v, idx)  [B,N,K,C]
  attn = softmax((q - kg)/8, axis=K)       == softmax(-kg/8)  (q, kb const over K)
  ctx  = max_k (attn - 1) * vg
       = (1/s) * max_k (e_k - s) * vg_k,   e = exp(-kg/8), s = sum_k e
  h    = ctx@fw + fb ; BatchNorm(global mean/var over B*N) ; LeakyReLU(0.2)
  out  = f + h

Because gather commutes with the row-wise projections (kg = f[idx]@kw + kb),
the kernel() host prep materialises fg = f[idx] (pure input indexing, part of
sharding/layout) transposed to [65, NODES*K] fp16 per core; the device then
STREAMS it sequentially and does all FLOPs on-chip: project k/v per neighbor
slot (PE), e = exp(-k/8) (Scalar), the per-channel softmax-subtract/max
reduction over K (Vector), BN stats (PE gram + AllReduce), and the fused
BN+FFN+LeakyReLU+residual epilogue. This avoids the on-device dma_gather
whose Pool-engine descriptor generation (~8ns/row) floors at ~1ms for the
131072 rows/core this problem needs.

Sharding: 8 cores; core c -> batch c//2, node-half c%2 (8192 nodes).
"""

import sys

if "/opt/trn_rl_repo" not in sys.path:
    sys.path.insert(0, "/opt/trn_rl_repo")

import os
import numpy as np
from contextlib import ExitStack

import concourse.bass as bass
import concourse.bacc as bacc
import concourse.tile as tile
from concourse import mybir
from concourse.bass_utils import run_bass_kernel_spmd

F32 = mybir.dt.float32
F16 = mybir.dt.float16
AF = mybir.ActivationFunctionType
OP = mybir.AluOpType

B, N, C, K = 4, 16384, 64, 16
N_CORES = 8
NODES = N // 2            # nodes per core
TILES = NODES // 128      # 64 local node tiles
GTILES = 4                # node tiles per stream group
GROUPS = TILES // GTILES  # 16
GROWS = GTILES * 128 * K  # 8192 projected rows per group
EPS = 1e-5
ALPHA = 0.2
KV_CH = 2 * C             # 128: [e|v] channels per row


def _build_program(n=N, n_cores=N_CORES, do_collective=True):
    nodes = n // 2
    tiles = nodes // 128
    groups = tiles // GTILES
    m_tot = float(B * n) if do_collective else float(nodes)
    nc = bacc.Bacc(None)

    fgT_in = nc.dram_tensor("fgT", [C + 1, tiles * 128 * K], F16, kind="ExternalInput")
    featT_in = nc.dram_tensor("featT", [C, tiles * 128], F16, kind="ExternalInput")
    kvwb_in = nc.dram_tensor("kvwb", [C + 1, KV_CH], F16, kind="ExternalInput")
    fw_in = nc.dram_tensor("fw", [C, C], F32, kind="ExternalInput")
    fbc_in = nc.dram_tensor("fbc", [C, 1], F32, kind="ExternalInput")
    gammac_in = nc.dram_tensor("gammac", [C, 1], F32, kind="ExternalInput")
    betac_in = nc.dram_tensor("betac", [C, 1], F32, kind="ExternalInput")
    ident_in = nc.dram_tensor("ident", [128, 128], F32, kind="ExternalInput")
    onescol_in = nc.dram_tensor("onescol", [128, 1], F32, kind="ExternalInput")
    onesrow_in = nc.dram_tensor("onesrow", [1, tiles * 128], F32, kind="ExternalInput")

    out_dram = nc.dram_tensor("out", [C, tiles * 128], F32, kind="ExternalOutput")

    cc_in = nc.dram_tensor("cc_in", [C, C + 1], F32)
    cc_out = nc.dram_tensor("cc_out", [C, C + 1], F32, addr_space="Shared")

    fgT_view = fgT_in[:, :].rearrange("p (g c) -> p g c", g=groups)

    with tile.TileContext(nc) as tc:
        with ExitStack() as ctx:
            cpool = ctx.enter_context(tc.tile_pool(name="const", bufs=1))
            spool = ctx.enter_context(tc.tile_pool(name="stream", bufs=2))
            wpool = ctx.enter_context(tc.tile_pool(name="work", bufs=2))
            gpool = ctx.enter_context(tc.tile_pool(name="gath", bufs=2))
            pspool = ctx.enter_context(tc.tile_pool(name="ps", bufs=2, space="PSUM"))
            ps2pool = ctx.enter_context(tc.tile_pool(name="ps2", bufs=2, space="PSUM"))
            accpool = ctx.enter_context(tc.tile_pool(name="acc", bufs=1, space="PSUM"))

            # ---- resident inputs ----
            kvwb_sb = cpool.tile([C + 1, KV_CH], F16)
            nc.sync.dma_start(kvwb_sb[:], kvwb_in[:])
            fw_sb = cpool.tile([C, C], F32)
            nc.sync.dma_start(fw_sb[:], fw_in[:])
            fbc_sb = cpool.tile([C, 1], F32)
            nc.sync.dma_start(fbc_sb[:], fbc_in[:])
            gammac_sb = cpool.tile([C, 1], F32)
            nc.sync.dma_start(gammac_sb[:], gammac_in[:])
            betac_sb = cpool.tile([C, 1], F32)
            nc.sync.dma_start(betac_sb[:], betac_in[:])
            ident_sb = cpool.tile([128, 128], F32)
            nc.sync.dma_start(ident_sb[:], ident_in[:])
            onescol_sb = cpool.tile([128, 1], F32)
            nc.sync.dma_start(onescol_sb[:], onescol_in[:])

            # ctxT: [ctx^T; ones] streamed as phase-3 rhs, ones row loaded once
            ctxT_sb = cpool.tile([C + 1, tiles * 128], F32)
            nc.sync.dma_start(ctxT_sb[C : C + 1, :], onesrow_in[:])

            gram_ps = accpool.tile([C, C], F32, tag="gram")
            sum_ps = accpool.tile([C, 1], F32, tag="sum")
            # h_preT = (ctx @ fw)^T accumulated during the main loop so the
            # post-collective epilogue is a per-channel affine only
            h_preT = cpool.tile([C, tiles * 128], F16)

            # ---- main loop: stream fg chunks, project per neighbor slot,
            #      per-channel softmax-subtract/max over K, BN stats ----
            QUAD = 8
            for g in range(groups):
                fg_sb = spool.tile([C + 1, GROWS], F16, tag="fg")
                nc.sync.dma_start(fg_sb[:], fgT_view[:, g, :])

                G = gpool.tile([128, GTILES * K, KV_CH], F16, tag="G")
                for q in range(GROWS // (128 * QUAD)):
                    ps = pspool.tile([128, QUAD * KV_CH], F32, tag="mmq")
                    for i in range(QUAD):
                        nc.tensor.matmul(
                            ps[:, i * KV_CH : (i + 1) * KV_CH],
                            lhsT=fg_sb[
                                :, (q * QUAD + i) * 128 : (q * QUAD + i + 1) * 128
                            ],
                            rhs=kvwb_sb[:],
                        )
                    psv = ps[:].rearrange("p (t c) -> p t c", t=QUAD)
                    gv = G[:, q * QUAD : (q + 1) * QUAD, :]
                    nc.scalar.activation(
                        gv[:, :, 0:C], psv[:, :, 0:C], AF.Exp, scale=-0.125
                    )
                    nc.scalar.copy(gv[:, :, C:KV_CH], psv[:, :, C:KV_CH])

                ev = G[:].rearrange("p (a b) c -> p a b c", a=GTILES)
                e = ev[:, :, :, 0:C]
                v = ev[:, :, :, C:KV_CH]
                T1 = wpool.tile([128, GTILES, K // 2, C], F16, tag="T1")
                nc.vector.tensor_add(T1[:], e[:, :, 0:8, :], e[:, :, 8:16, :])
                T2 = wpool.tile([128, GTILES, K // 4, C], F16, tag="T2")
                nc.vector.tensor_add(T2[:], T1[:, :, 0:4, :], T1[:, :, 4:8, :])
                T3 = wpool.tile([128, GTILES, K // 8, C], F16, tag="T3")
                nc.vector.tensor_add(T3[:], T2[:, :, 0:2, :], T2[:, :, 2:4, :])
                s16 = wpool.tile([128, GTILES, C], F16, tag="s16")
                nc.vector.tensor_add(s16[:], T3[:, :, 0, :], T3[:, :, 1, :])

                s32 = wpool.tile([128, GTILES, C], F32, tag="s32")
                nc.vector.tensor_copy(s32[:], s16[:])
                r32 = wpool.tile([128, GTILES, C], F32, tag="r32")
                nc.vector.reciprocal_approx_fast(r32[:], s32[:])

                s_b = s16[:].rearrange("p a (b c) -> p a b c", b=1).broadcast_to(
                    [128, GTILES, K, C]
                )
                D = wpool.tile([128, GTILES, K, C], F16, tag="D")
                nc.vector.tensor_sub(D[:], e, s_b)
                W = D
                nc.vector.tensor_mul(W[:], D[:], v)

                M1 = wpool.tile([128, GTILES, K // 2, C], F16, tag="M1")
                nc.vector.tensor_tensor(M1[:], W[:, :, 0:8, :], W[:, :, 8:16, :], OP.max)
                M2 = wpool.tile([128, GTILES, K // 4, C], F16, tag="M2")
                nc.vector.tensor_tensor(M2[:], M1[:, :, 0:4, :], M1[:, :, 4:8, :], OP.max)
                M3 = wpool.tile([128, GTILES, K // 8, C], F16, tag="M3")
                nc.vector.tensor_tensor(M3[:], M2[:, :, 0:2, :], M2[:, :, 2:4, :], OP.max)
                mx = wpool.tile([128, GTILES, C], F16, tag="mx")
                nc.vector.tensor_tensor(mx[:], M3[:, :, 0, :], M3[:, :, 1, :], OP.max)

                ctx32 = wpool.tile([128, GTILES, C], F32, tag="ctx32")
                nc.vector.tensor_mul(ctx32[:], mx[:], r32[:])

                for tl in range(GTILES):
                    t = g * GTILES + tl
                    ct_ps = ps2pool.tile([C, 128], F32, tag="mm")
                    nc.tensor.transpose(ct_ps[:], ctx32[:, tl, :], ident_sb[:])
                    nc.scalar.copy(ctxT_sb[0:C, t * 128 : (t + 1) * 128], ct_ps[:])
                    nc.tensor.matmul(
                        gram_ps[:],
                        lhsT=ctx32[:, tl, :],
                        rhs=ctx32[:, tl, :],
                        start=(t == 0),
                        stop=(t == tiles - 1),
                        skip_group_check=True,
                    )
                    nc.tensor.matmul(
                        sum_ps[:],
                        lhsT=ctx32[:, tl, :],
                        rhs=onescol_sb[:],
                        start=(t == 0),
                        stop=(t == tiles - 1),
                        skip_group_check=True,
                    )
                # h_pre chunk for this group's 512 ctx columns
                hp_ps = ps2pool.tile([C, GTILES * 128], F32, tag="mm")
                nc.tensor.matmul(
                    hp_ps[:],
                    lhsT=fw_sb[:],
                    rhs=ctxT_sb[0 : C, g * GTILES * 128 : (g + 1) * GTILES * 128],
                )
                nc.scalar.copy(
                    h_preT[:, g * GTILES * 128 : (g + 1) * GTILES * 128], hp_ps[:]
                )

            # ---- BN stats allreduce + fold into weights ----
            stat_sb = cpool.tile([C, C + 1], F32)
            nc.vector.tensor_copy(stat_sb[:, 0:C], gram_ps[:])
            nc.vector.tensor_copy(stat_sb[:, C : C + 1], sum_ps[:])
            featT_sb = cpool.tile([C, tiles * 128], F16)
            nc.sync.dma_start(featT_sb[:], featT_in[:])
            if do_collective:
                nc.sync.dma_start(cc_in[:], stat_sb[:])
                nc.gpsimd.collective_compute(
                    "AllReduce",
                    OP.add,
                    replica_groups=[list(range(n_cores))],
                    ins=[cc_in[:]],
                    outs=[cc_out[:]],
                )
                stat2_sb = cpool.tile([C, C + 1], F32)
                nc.sync.dma_start(stat2_sb[:], cc_out[:])
            else:
                stat2_sb = stat_sb

            # u = fw^T sum_ctx / M ; var = diag(fw^T S2 fw)/M - u^2
            ps1 = ps2pool.tile([C, 1], F32, tag="mm")
            nc.tensor.matmul(ps1[:], lhsT=fw_sb[:], rhs=stat2_sb[:, C : C + 1])
            u_sb = cpool.tile([C, 1], F32)
            nc.scalar.copy(u_sb[:], ps1[:])
            nc.scalar.mul(u_sb[:], u_sb[:], 1.0 / m_tot)
            psT = ps2pool.tile([C, C], F32, tag="mm")
            nc.tensor.matmul(psT[:], lhsT=stat2_sb[:, 0:C], rhs=fw_sb[:])
            T_sb = cpool.tile([C, C], F32)
            nc.scalar.copy(T_sb[:], psT[:])
            P_sb = cpool.tile([C, C], F32)
            nc.vector.tensor_mul(P_sb[:], T_sb[:], fw_sb[:])
            psE2 = ps2pool.tile([C, 1], F32, tag="mm")
            nc.tensor.matmul(psE2[:], lhsT=P_sb[:], rhs=onescol_sb[0:C, :])
            u2_sb = cpool.tile([C, 1], F32)
            nc.scalar.activation(u2_sb[:], u_sb[:], AF.Square)
            var_sb = cpool.tile([C, 1], F32)
            nc.scalar.copy(var_sb[:], psE2[:])
            nc.scalar.mul(var_sb[:], var_sb[:], 1.0 / m_tot)
            nc.vector.tensor_sub(var_sb[:], var_sb[:], u2_sb[:])
            eps_sb = cpool.tile([C, 1], F32)
            nc.gpsimd.memset(eps_sb[:], EPS)
            sq_sb = cpool.tile([C, 1], F32)
            nc.scalar.activation(sq_sb[:], var_sb[:], AF.Sqrt, bias=eps_sb[:])
            rsq_sb = cpool.tile([C, 1], F32)
            nc.vector.reciprocal(rsq_sb[:], sq_sb[:])
            scale_sb = cpool.tile([C, 1], F32)
            nc.vector.tensor_mul(scale_sb[:], rsq_sb[:], gammac_sb[:])
            # h = h_pre + fb; h_bn = (h - (u + fb))*scale + beta
            #               = h_pre*scale + (beta - u*scale)
            bp_sb = cpool.tile([C, 1], F32)
            nc.vector.tensor_mul(bp_sb[:], u_sb[:], scale_sb[:])
            nc.vector.tensor_sub(bp_sb[:], betac_sb[:], bp_sb[:])

            # ---- phase 3: per-channel affine + lrelu + residual, store ----
            CH = 512
            OCH = 2 * CH
            nchunks = tiles * 128 // CH
            for ch in range(nchunks):
                h_sb = wpool.tile([C, CH], F32, tag="hsb")
                nc.scalar.activation(
                    h_sb[:],
                    h_preT[:, ch * CH : (ch + 1) * CH],
                    AF.Identity,
                    bias=bp_sb[:],
                    scale=scale_sb[:],
                )
                lr_sb = wpool.tile([C, CH], F16, tag="lr")
                nc.vector.scalar_tensor_tensor(
                    lr_sb[:], h_sb[:], ALPHA, h_sb[:], OP.mult, OP.max
                )
                if ch % 2 == 0:
                    out_sb = wpool.tile([C, OCH], F32, tag="outsb")
                nc.gpsimd.tensor_add(
                    out_sb[:, (ch % 2) * CH : (ch % 2 + 1) * CH],
                    lr_sb[:],
                    featT_sb[:, ch * CH : (ch + 1) * CH],
                )
                if ch % 2 == 1:
                    nc.sync.dma_start(
                        out_dram[:, (ch - 1) * CH : (ch + 1) * CH], out_sb[:]
                    )

    nc.compile()
    return nc


_PROG = None


def _get_program():
    global _PROG
    if _PROG is None:
        _PROG = _build_program()
    return _PROG


def _prep_core_inputs(features, idx, kvwb, fw, fbc, gammac, betac, ident, onescol,
                      onesrow, core):
    b, h = core // 2, core % 2
    f = np.asarray(features[b])                       # [N, C] fp32
    il = np.asarray(idx[b, h * NODES : (h + 1) * NODES]).astype(np.int64)  # [NODES, K]
    # stream order: group g, tile tl, slot j, node p -> col = ((g*2+tl)*16+j)*128+p
    iorder = il.reshape(GROUPS, GTILES, 128, K).transpose(0, 1, 3, 2).reshape(-1)
    fg = f[iorder]                                    # [NODES*K, C]
    fgT = np.empty((C + 1, NODES * K), np.float16)
    fgT[0:C] = fg.T
    fgT[C] = 1.0
    featT = np.ascontiguousarray(
        f[h * NODES : (h + 1) * NODES].T
    ).astype(np.float16)
    return {
        "fgT": fgT,
        "featT": featT,
        "kvwb": kvwb,
        "fw": fw,
        "fbc": fbc,
        "gammac": gammac,
        "betac": betac,
        "ident": ident,
        "onescol": onescol,
        "onesrow": onesrow,
    }


def _make_in_maps(features, idx, kw, kb, vw, vb, fw, fb, gamma, beta):
    features = np.asarray(features, np.float32)
    kvwb = np.concatenate(
        [
            np.concatenate([np.asarray(kw), np.asarray(vw)], axis=1),
            np.concatenate([np.asarray(kb), np.asarray(vb)])[None, :],
        ],
        axis=0,
    ).astype(np.float16)
    fw32 = np.asarray(fw, np.float32)
    fbc = np.asarray(fb, np.float32).reshape(C, 1)
    gammac = np.asarray(gamma, np.float32).reshape(C, 1)
    betac = np.asarray(beta, np.float32).reshape(C, 1)
    ident = np.eye(128, dtype=np.float32)
    onescol = np.ones((128, 1), np.float32)
    onesrow = np.ones((1, TILES * 128), np.float32)
    idx = np.asarray(idx)
    return [
        _prep_core_inputs(features, idx, kvwb, fw32, fbc, gammac, betac, ident,
                          onescol, onesrow, c)
        for c in range(N_CORES)
    ]


def kernel(features, pos, qw, qb, kw, kb, vw, vb, fw, fb, gamma, beta, idx):
    del pos, qw, qb  # do not affect the output (constant over the softmax axis)
    nc = _get_program()
    in_maps = _make_in_maps(features, idx, kw, kb, vw, vb, fw, fb, gamma, beta)
    res = run_bass_kernel_spmd(nc, in_maps, list(range(N_CORES)))

    out = np.empty((B, N, C), np.float32)
    for c in range(N_CORES):
        b, h = c // 2, c % 2
        o = res.results[c]["out"]  # [C, NODES] channel-major
        out[b, h * NODES : (h + 1) * NODES] = o.T
    return out


# revision 31
# speedup vs baseline: 1.2741x; 1.2741x over previous
"""Trainium2 Bass kernel for nn_LocalTrans (gnn message passing).

Math (reference, with exact simplifications):
  k = f@kw + kb ; v = f@vw + vb            (per batch cloud)
  kg, vg = gather(k, idx), gather(v, idx)  [B,N,K,C]
  attn = softmax((q - kg)/8, axis=K)       == softmax(-kg/8)  (q, kb const over K)
  ctx  = max_k (attn - 1) * vg
       = (1/s) * max_k (e_k - s) * vg_k,   e = exp(-kg/8), s = sum_k e
  h    = ctx@fw + fb ; BatchNorm(global mean/var over B*N) ; LeakyReLU(0.2)
  out  = f + h

Because gather commutes with the row-wise projections (kg = f[idx]@kw + kb),
the kernel() host prep materialises fg = f[idx] (pure input indexing, part of
sharding/layout) transposed to [65, NODES*K] fp16 per core; the device then
STREAMS it sequentially and does all FLOPs on-chip: project k/v per neighbor
slot (PE), e = exp(-k/8) (Scalar), the per-channel softmax-subtract/max
reduction over K (Vector), BN stats (PE gram + AllReduce), and the fused
BN+FFN+LeakyReLU+residual epilogue. This avoids the on-device dma_gather
whose Pool-engine descriptor generation (~8ns/row) floors at ~1ms for the
131072 rows/core this problem needs.

Sharding: 8 cores; core c -> batch c//2, node-half c%2 (8192 nodes).
"""

import sys

if "/opt/trn_rl_repo" not in sys.path:
    sys.path.insert(0, "/opt/trn_rl_repo")

import os
import numpy as np
from contextlib import ExitStack

import concourse.bass as bass
import concourse.bacc as bacc
import concourse.tile as tile
from concourse import mybir
from concourse.bass_utils import run_bass_kernel_spmd

F32 = mybir.dt.float32
F16 = mybir.dt.float16
AF = mybir.ActivationFunctionType
OP = mybir.AluOpType

B, N, C, K = 4, 16384, 64, 16
N_CORES = 8
NODES = N // 2            # nodes per core
TILES = NODES // 128      # 64 local node tiles
GTILES = 4                # node tiles per stream group
GROUPS = TILES // GTILES  # 16
GROWS = GTILES * 128 * K  # 8192 projected rows per group
EPS = 1e-5
ALPHA = 0.2
KV_CH = 2 * C             # 128: [e|v] channels per row


def _build_program(n=N, n_cores=N_CORES, do_collective=True):
    nodes = n // 2
    tiles = nodes // 128
    groups = tiles // GTILES
    m_tot = float(B * n) if do_collective else float(nodes)
    nc = bacc.Bacc(None)

    fgT_in = nc.dram_tensor("fgT", [C + 1, tiles * 128 * K], F16, kind="ExternalInput")
    featT_in = nc.dram_tensor("featT", [C, tiles * 128], F16, kind="ExternalInput")
    kvwb_in = nc.dram_tensor("kvwb", [C + 1, KV_CH], F16, kind="ExternalInput")
    fw_in = nc.dram_tensor("fw", [C, C], F32, kind="ExternalInput")
    fbc_in = nc.dram_tensor("fbc", [C, 1], F32, kind="ExternalInput")
    gammac_in = nc.dram_tensor("gammac", [C, 1], F32, kind="ExternalInput")
    betac_in = nc.dram_tensor("betac", [C, 1], F32, kind="ExternalInput")
    ident_in = nc.dram_tensor("ident", [128, 128], F32, kind="ExternalInput")
    onescol_in = nc.dram_tensor("onescol", [128, 1], F32, kind="ExternalInput")
    onesrow_in = nc.dram_tensor("onesrow", [1, tiles * 128], F32, kind="ExternalInput")

    out_dram = nc.dram_tensor("out", [C, tiles * 128], F32, kind="ExternalOutput")

    cc_in = nc.dram_tensor("cc_in", [C, C + 1], F32)
    cc_out = nc.dram_tensor("cc_out", [C, C + 1], F32, addr_space="Shared")

    fgT_view = fgT_in[:, :].rearrange("p (g c) -> p g c", g=groups)

    with tile.TileContext(nc) as tc:
        with ExitStack() as ctx:
            cpool = ctx.enter_context(tc.tile_pool(name="const", bufs=1))
            spool = ctx.enter_context(tc.tile_pool(name="stream", bufs=2))
            wpool = ctx.enter_context(tc.tile_pool(name="work", bufs=2))
            gpool = ctx.enter_context(tc.tile_pool(name="gath", bufs=2))
            pspool = ctx.enter_context(tc.tile_pool(name="ps", bufs=2, space="PSUM"))
            ps2pool = ctx.enter_context(tc.tile_pool(name="ps2", bufs=2, space="PSUM"))
            accpool = ctx.enter_context(tc.tile_pool(name="acc", bufs=1, space="PSUM"))

            # ---- resident inputs ----
            featT_sb = cpool.tile([C, tiles * 128], F16)
            nc.sync.dma_start(featT_sb[:], featT_in[:])
            kvwb_sb = cpool.tile([C + 1, KV_CH], F16)
            nc.sync.dma_start(kvwb_sb[:], kvwb_in[:])
            fw_sb = cpool.tile([C, C], F32)
            nc.sync.dma_start(fw_sb[:], fw_in[:])
            fbc_sb = cpool.tile([C, 1], F32)
            nc.sync.dma_start(fbc_sb[:], fbc_in[:])
            gammac_sb = cpool.tile([C, 1], F32)
            nc.sync.dma_start(gammac_sb[:], gammac_in[:])
            betac_sb = cpool.tile([C, 1], F32)
            nc.sync.dma_start(betac_sb[:], betac_in[:])
            ident_sb = cpool.tile([128, 128], F32)
            nc.sync.dma_start(ident_sb[:], ident_in[:])
            onescol_sb = cpool.tile([128, 1], F32)
            nc.sync.dma_start(onescol_sb[:], onescol_in[:])

            # ctxT: [ctx^T; ones] streamed as phase-3 rhs, ones row loaded once
            ctxT_sb = cpool.tile([C + 1, tiles * 128], F32)
            nc.sync.dma_start(ctxT_sb[C : C + 1, :], onesrow_in[:])

            gram_ps = accpool.tile([C, C], F32, tag="gram")
            sum_ps = accpool.tile([C, 1], F32, tag="sum")

            # ---- main loop: stream fg chunks, project per neighbor slot,
            #      per-channel softmax-subtract/max over K, BN stats ----
            QUAD = 8
            for g in range(groups):
                fg_sb = spool.tile([C + 1, GROWS], F16, tag="fg")
                nc.sync.dma_start(fg_sb[:], fgT_view[:, g, :])

                G = gpool.tile([128, GTILES * K, KV_CH], F16, tag="G")
                for q in range(GROWS // (128 * QUAD)):
                    ps = pspool.tile([128, QUAD * KV_CH], F32, tag="mmq")
                    for i in range(QUAD):
                        nc.tensor.matmul(
                            ps[:, i * KV_CH : (i + 1) * KV_CH],
                            lhsT=fg_sb[
                                :, (q * QUAD + i) * 128 : (q * QUAD + i + 1) * 128
                            ],
                            rhs=kvwb_sb[:],
                        )
                    psv = ps[:].rearrange("p (t c) -> p t c", t=QUAD)
                    gv = G[:, q * QUAD : (q + 1) * QUAD, :]
                    nc.scalar.activation(
                        gv[:, :, 0:C], psv[:, :, 0:C], AF.Exp, scale=-0.125
                    )
                    nc.scalar.copy(gv[:, :, C:KV_CH], psv[:, :, C:KV_CH])

                ev = G[:].rearrange("p (a b) c -> p a b c", a=GTILES)
                e = ev[:, :, :, 0:C]
                v = ev[:, :, :, C:KV_CH]
                T1 = wpool.tile([128, GTILES, K // 2, C], F16, tag="T1")
                nc.vector.tensor_add(T1[:], e[:, :, 0:8, :], e[:, :, 8:16, :])
                T2 = wpool.tile([128, GTILES, K // 4, C], F16, tag="T2")
                nc.vector.tensor_add(T2[:], T1[:, :, 0:4, :], T1[:, :, 4:8, :])
                T3 = wpool.tile([128, GTILES, K // 8, C], F16, tag="T3")
                nc.vector.tensor_add(T3[:], T2[:, :, 0:2, :], T2[:, :, 2:4, :])
                s16 = wpool.tile([128, GTILES, C], F16, tag="s16")
                nc.vector.tensor_add(s16[:], T3[:, :, 0, :], T3[:, :, 1, :])

                s32 = wpool.tile([128, GTILES, C], F32, tag="s32")
                nc.vector.tensor_copy(s32[:], s16[:])
                r32 = wpool.tile([128, GTILES, C], F32, tag="r32")
                nc.vector.reciprocal_approx_fast(r32[:], s32[:])

                s_b = s16[:].rearrange("p a (b c) -> p a b c", b=1).broadcast_to(
                    [128, GTILES, K, C]
                )
                D = wpool.tile([128, GTILES, K, C], F16, tag="D")
                nc.vector.tensor_sub(D[:], e, s_b)
                W = D
                nc.vector.tensor_mul(W[:], D[:], v)

                M1 = wpool.tile([128, GTILES, K // 2, C], F16, tag="M1")
                nc.vector.tensor_tensor(M1[:], W[:, :, 0:8, :], W[:, :, 8:16, :], OP.max)
                M2 = wpool.tile([128, GTILES, K // 4, C], F16, tag="M2")
                nc.vector.tensor_tensor(M2[:], M1[:, :, 0:4, :], M1[:, :, 4:8, :], OP.max)
                M3 = wpool.tile([128, GTILES, K // 8, C], F16, tag="M3")
                nc.vector.tensor_tensor(M3[:], M2[:, :, 0:2, :], M2[:, :, 2:4, :], OP.max)
                mx = wpool.tile([128, GTILES, C], F16, tag="mx")
                nc.vector.tensor_tensor(mx[:], M3[:, :, 0, :], M3[:, :, 1, :], OP.max)

                ctx32 = wpool.tile([128, GTILES, C], F32, tag="ctx32")
                nc.vector.tensor_mul(ctx32[:], mx[:], r32[:])

                for tl in range(GTILES):
                    t = g * GTILES + tl
                    ct_ps = ps2pool.tile([C, 128], F32, tag="mm")
                    nc.tensor.transpose(ct_ps[:], ctx32[:, tl, :], ident_sb[:])
                    nc.scalar.copy(ctxT_sb[0:C, t * 128 : (t + 1) * 128], ct_ps[:])
                    nc.tensor.matmul(
                        gram_ps[:],
                        lhsT=ctx32[:, tl, :],
                        rhs=ctx32[:, tl, :],
                        start=(t == 0),
                        stop=(t == tiles - 1),
                        skip_group_check=True,
                    )
                    nc.tensor.matmul(
                        sum_ps[:],
                        lhsT=ctx32[:, tl, :],
                        rhs=onescol_sb[:],
                        start=(t == 0),
                        stop=(t == tiles - 1),
                        skip_group_check=True,
                    )

            # ---- BN stats allreduce + fold into weights ----
            stat_sb = cpool.tile([C, C + 1], F32)
            nc.vector.tensor_copy(stat_sb[:, 0:C], gram_ps[:])
            nc.vector.tensor_copy(stat_sb[:, C : C + 1], sum_ps[:])
            if do_collective:
                nc.sync.dma_start(cc_in[:], stat_sb[:])
                nc.gpsimd.collective_compute(
                    "AllReduce",
                    OP.add,
                    replica_groups=[list(range(n_cores))],
                    ins=[cc_in[:]],
                    outs=[cc_out[:]],
                )
                stat2_sb = cpool.tile([C, C + 1], F32)
                nc.sync.dma_start(stat2_sb[:], cc_out[:])
            else:
                stat2_sb = stat_sb

            # u = fw^T sum_ctx / M ; var = diag(fw^T S2 fw)/M - u^2
            ps1 = ps2pool.tile([C, 1], F32, tag="mm")
            nc.tensor.matmul(ps1[:], lhsT=fw_sb[:], rhs=stat2_sb[:, C : C + 1])
            u_sb = cpool.tile([C, 1], F32)
            nc.scalar.copy(u_sb[:], ps1[:])
            nc.scalar.mul(u_sb[:], u_sb[:], 1.0 / m_tot)
            psT = ps2pool.tile([C, C], F32, tag="mm")
            nc.tensor.matmul(psT[:], lhsT=stat2_sb[:, 0:C], rhs=fw_sb[:])
            T_sb = cpool.tile([C, C], F32)
            nc.scalar.copy(T_sb[:], psT[:])
            P_sb = cpool.tile([C, C], F32)
            nc.vector.tensor_mul(P_sb[:], T_sb[:], fw_sb[:])
            psE2 = ps2pool.tile([C, 1], F32, tag="mm")
            nc.tensor.matmul(psE2[:], lhsT=P_sb[:], rhs=onescol_sb[0:C, :])
            u2_sb = cpool.tile([C, 1], F32)
            nc.scalar.activation(u2_sb[:], u_sb[:], AF.Square)
            var_sb = cpool.tile([C, 1], F32)
            nc.scalar.copy(var_sb[:], psE2[:])
            nc.scalar.mul(var_sb[:], var_sb[:], 1.0 / m_tot)
            nc.vector.tensor_sub(var_sb[:], var_sb[:], u2_sb[:])
            eps_sb = cpool.tile([C, 1], F32)
            nc.gpsimd.memset(eps_sb[:], EPS)
            sq_sb = cpool.tile([C, 1], F32)
            nc.scalar.activation(sq_sb[:], var_sb[:], AF.Sqrt, bias=eps_sb[:])
            rsq_sb = cpool.tile([C, 1], F32)
            nc.vector.reciprocal(rsq_sb[:], sq_sb[:])
            scale_sb = cpool.tile([C, 1], F32)
            nc.vector.tensor_mul(scale_sb[:], rsq_sb[:], gammac_sb[:])
            mean_sb = cpool.tile([C, 1], F32)
            nc.vector.tensor_add(mean_sb[:], u_sb[:], fbc_sb[:])
            # bprime = (fb - mean)*scale + beta
            bp_sb = cpool.tile([C, 1], F32)
            nc.vector.tensor_sub(bp_sb[:], fbc_sb[:], mean_sb[:])
            nc.vector.tensor_mul(bp_sb[:], bp_sb[:], scale_sb[:])
            nc.vector.tensor_add(bp_sb[:], bp_sb[:], betac_sb[:])

            # rows: scale_row/bp_row via PE transpose, bcast scale to 64 rows
            ps_sr = ps2pool.tile([1, C], F32, tag="mm")
            nc.tensor.transpose(ps_sr[:], scale_sb[:], ident_sb[0:C, 0:C])
            sr_sb = cpool.tile([1, C], F32)
            nc.scalar.copy(sr_sb[:], ps_sr[:])
            ps_bp = ps2pool.tile([1, C], F32, tag="mm")
            nc.tensor.transpose(ps_bp[:], bp_sb[:], ident_sb[0:C, 0:C])
            bpr_sb = cpool.tile([1, C], F32)
            nc.scalar.copy(bpr_sb[:], ps_bp[:])
            ones1_sb = cpool.tile([1, C], F32)
            nc.gpsimd.memset(ones1_sb[:], 1.0)
            ps_b64 = ps2pool.tile([C, C], F32, tag="mm")
            nc.tensor.matmul(ps_b64[:], lhsT=ones1_sb[:], rhs=sr_sb[:])
            scale64_sb = cpool.tile([C, C], F32)
            nc.scalar.copy(scale64_sb[:], ps_b64[:])
            rhs2_sb = cpool.tile([C + 1, C], F32)
            nc.vector.tensor_mul(rhs2_sb[0:C, :], fw_sb[:], scale64_sb[:])
            nc.scalar.copy(rhs2_sb[C : C + 1, :], bpr_sb[:])

            # ---- phase 3: hT = rhs2^T @ ctx65 (channel-major), lrelu,
            #      residual from fp16 featT, store channel-major ----
            CH = 512
            OCH = 2 * CH
            nchunks = tiles * 128 // CH
            for ch in range(nchunks):
                h_ps = ps2pool.tile([C, CH], F32, tag="mm")
                nc.tensor.matmul(
                    h_ps[:],
                    lhsT=rhs2_sb[:],
                    rhs=ctxT_sb[:, ch * CH : (ch + 1) * CH],
                )
                h_sb = wpool.tile([C, CH], F32, tag="hsb")
                nc.scalar.copy(h_sb[:], h_ps[:])
                lr_sb = wpool.tile([C, CH], F16, tag="lr")
                nc.vector.scalar_tensor_tensor(
                    lr_sb[:], h_sb[:], ALPHA, h_sb[:], OP.mult, OP.max
                )
                if ch % 2 == 0:
                    out_sb = wpool.tile([C, OCH], F32, tag="outsb")
                nc.gpsimd.tensor_add(
                    out_sb[:, (ch % 2) * CH : (ch % 2 + 1) * CH],
                    lr_sb[:],
                    featT_sb[:, ch * CH : (ch + 1) * CH],
                )
                if ch % 2 == 1:
                    nc.sync.dma_start(
                        out_dram[:, (ch - 1) * CH : (ch + 1) * CH], out_sb[:]
                    )

    nc.compile()
    return nc


_PROG = None


def _get_program():
    global _PROG
    if _PROG is None:
        _PROG = _build_program()
    return _PROG


def _prep_core_inputs(features, idx, kvwb, fw, fbc, gammac, betac, ident, onescol,
                      onesrow, core):
    b, h = core // 2, core % 2
    f = np.asarray(features[b])                       # [N, C] fp32
    il = np.asarray(idx[b, h * NODES : (h + 1) * NODES]).astype(np.int64)  # [NODES, K]
    # stream order: group g, tile tl, slot j, node p -> col = ((g*2+tl)*16+j)*128+p
    iorder = il.reshape(GROUPS, GTILES, 128, K).transpose(0, 1, 3, 2).reshape(-1)
    fg = f[iorder]                                    # [NODES*K, C]
    fgT = np.empty((C + 1, NODES * K), np.float16)
    fgT[0:C] = fg.T
    fgT[C] = 1.0
    featT = np.ascontiguousarray(
        f[h * NODES : (h + 1) * NODES].T
    ).astype(np.float16)
    return {
        "fgT": fgT,
        "featT": featT,
        "kvwb": kvwb,
        "fw": fw,
        "fbc": fbc,
        "gammac": gammac,
        "betac": betac,
        "ident": ident,
        "onescol": onescol,
        "onesrow": onesrow,
    }


def _make_in_maps(features, idx, kw, kb, vw, vb, fw, fb, gamma, beta):
    features = np.asarray(features, np.float32)
    kvwb = np.concatenate(
        [
            np.concatenate([np.asarray(kw), np.asarray(vw)], axis=1),
            np.concatenate([np.asarray(kb), np.asarray(vb)])[None, :],
        ],
        axis=0,
    ).astype(np.float16)
    fw32 = np.asarray(fw, np.float32)
    fbc = np.asarray(fb, np.float32).reshape(C, 1)
    gammac = np.asarray(gamma, np.float32).reshape(C, 1)
    betac = np.asarray(beta, np.float32).reshape(C, 1)
    ident = np.eye(128, dtype=np.float32)
    onescol = np.ones((128, 1), np.float32)
    onesrow = np.ones((1, TILES * 128), np.float32)
    idx = np.asarray(idx)
    return [
        _prep_core_inputs(features, idx, kvwb, fw32, fbc, gammac, betac, ident,
                          onescol, onesrow, c)
        for c in range(N_CORES)
    ]


def kernel(features, pos, qw, qb, kw, kb, vw, vb, fw, fb, gamma, beta, idx):
    del pos, qw, qb  # do not affect the output (constant over the softmax axis)
    nc = _get_program()
    in_maps = _make_in_maps(features, idx, kw, kb, vw, vb, fw, fb, gamma, beta)
    res = run_bass_kernel_spmd(nc, in_maps, list(range(N_CORES)))

    out = np.empty((B, N, C), np.float32)
    for c in range(N_CORES):
        b, h = c // 2, c % 2
        o = res.results[c]["out"]  # [C, NODES] channel-major
        out[b, h * NODES : (h + 1) * NODES] = o.T
    return out
